# revision 5
# baseline (speedup 1.0000x reference)
"""CascadeMemoryAttention Trainium2 kernel.

Distribution over 8 NeuronCores: core = b*4 + g handles batch b (of 2) and
head group g (4 heads, 256 q/k/v channels). Per core, everything is computed
in a transposed ("channels-major") layout so that no on-device transposes are
needed anywhere:

  qT  [256,2048] = Wq_loc^T x^T          (host supplies x^T; Wq pre-scaled 1/sqrt(d))
  kT  [256,2560] = Wk_loc^T [x;fm;rm]^T
  v   [2560,256] keys-major (natural matmul output orientation)
  sT  [keys,tok] = kT_h^T qT_h per head  -> exp -> eT (bf16)
  yT  [64,tok]   = [v_h | 1]^T eT        (ones column gives Z = sum(e) for free)
  y   = (y_chunk + g * y_mem) / Z        (gate fused via broadcast rows)
  conv/residual in channels-major, then out_partial = y_tot^T Wo_loc and the
  host sums the 4 partials per batch.

Softmax max-subtraction is skipped: scores are ~N(0, 0.17), |s| < ~6, exp is
safe in fp32/bf16 and softmax is shift-invariant, so results are identical.
"""

import sys

sys.path.insert(0, "/opt/trn_rl_repo")

from contextlib import ExitStack

import numpy as np
import ml_dtypes

import concourse.bass as bass  # noqa: F401  (bass types reachable via bacc)
import concourse.mybir as mybir
import concourse.tile as tile
from concourse import bacc
from concourse.bass_utils import run_bass_kernel_spmd

F32 = mybir.dt.float32
BF16 = mybir.dt.bfloat16
AF = mybir.ActivationFunctionType
ALU = mybir.AluOpType

# Problem dims (hardcoded per the task contract).
B, T, C, H, M = 2, 2048, 1024, 16, 256
D = 64          # head dim
HL = 4          # heads per core
CL = HL * D     # 256 local channels
S = T + 2 * M   # 2560 keys
KT = 8          # feature k-tiles of 128
NKT = S // 128  # 20 key tiles (16 chunk + 4 memory)
NQB = T // 512  # 4 query blocks of 512 tokens
KC = 4          # canon conv taps
YW = T + 3      # yT row width incl. 3-col zero pad for causal conv


def build_nc():
    nc = bacc.Bacc("TRN2", target_bir_lowering=False, debug=False)

    xmT_d = nc.dram_tensor("xmT", [C, S], BF16, kind="ExternalInput")
    wq_d = nc.dram_tensor("wq", [C, CL], BF16, kind="ExternalInput")
    wk_d = nc.dram_tensor("wk", [C, CL], BF16, kind="ExternalInput")
    wv_d = nc.dram_tensor("wv", [C, CL], BF16, kind="ExternalInput")
    wo_d = nc.dram_tensor("wo", [CL, C], BF16, kind="ExternalInput")
    wg_d = nc.dram_tensor("wg", [C, HL], BF16, kind="ExternalInput")
    gb_d = nc.dram_tensor("gb", [HL, 1], F32, kind="ExternalInput")
    cw_d = nc.dram_tensor("cw", [CL, KC], F32, kind="ExternalInput")
    cb_d = nc.dram_tensor("cb", [CL, 1], F32, kind="ExternalInput")
    out_d = nc.dram_tensor("out", [T, C], F32, kind="ExternalOutput")

    with ExitStack() as ctx:
        tc = ctx.enter_context(tile.TileContext(nc))
        pp = ctx.enter_context(tc.tile_pool(name="persist", bufs=1))

        # ---- persistent SBUF tensors ----
        xm = pp.tile([128, KT * S], BF16, tag="xm")        # feature k-tile kt at [kt*S:]
        wq = pp.tile([128, KT * CL], BF16, tag="wq")
        wk = pp.tile([128, KT * CL], BF16, tag="wk")
        wv = pp.tile([128, KT * CL], BF16, tag="wv")
        wg = pp.tile([128, KT * HL], BF16, tag="wg")
        wo = pp.tile([128, 2 * C], BF16, tag="wo")         # ctile ct at [ct*C:]
        gbb = pp.tile([HL, 1], F32, tag="gbb")
        cw = pp.tile([128, 2 * KC], F32, tag="cw")
        cb = pp.tile([128, 2], F32, tag="cb")
        qT = pp.tile([128, 2 * T], BF16, tag="qT")         # m-tile at [m*T:]
        kT = pp.tile([128, 2 * S], BF16, tag="kT")
        vsb = pp.tile([128, NKT * (HL * 65)], BF16, tag="v")  # per key tile: 4x[64 v | 1]
        g4 = pp.tile([HL, T], F32, tag="g4")               # sigmoid gates, partition=head
        gfl = pp.tile([1, HL * T], F32, tag="gfl")         # gates flattened to partition 0
        mB = pp.tile([128, 1024], BF16, tag="mB")          # sliding causal mask
        yT = pp.tile([128, 2 * YW], F32, tag="yT")         # attention out, ch-major
        ytot = pp.tile([128, 2 * T], BF16, tag="ytot")     # after conv+residual

        # ---- input DMAs ----
        nc.sync.dma_start(xm[:].rearrange("p (kt s) -> p kt s", kt=KT),
                          xmT_d[:].rearrange("(kt p) s -> p kt s", p=128))
        nc.sync.dma_start(wq[:].rearrange("p (kt c) -> p kt c", kt=KT),
                          wq_d[:].rearrange("(kt p) c -> p kt c", p=128))
        nc.sync.dma_start(wk[:].rearrange("p (kt c) -> p kt c", kt=KT),
                          wk_d[:].rearrange("(kt p) c -> p kt c", p=128))
        nc.sync.dma_start(wv[:].rearrange("p (kt c) -> p kt c", kt=KT),
                          wv_d[:].rearrange("(kt p) c -> p kt c", p=128))
        nc.sync.dma_start(wg[:].rearrange("p (kt h) -> p kt h", kt=KT),
                          wg_d[:].rearrange("(kt p) h -> p kt h", p=128))
        nc.sync.dma_start(wo[:].rearrange("p (ct n) -> p ct n", ct=2),
                          wo_d[:].rearrange("(ct p) n -> p ct n", p=128))
        nc.sync.dma_start(gbb[:], gb_d[:])
        nc.sync.dma_start(cw[:].rearrange("p (ct k) -> p ct k", ct=2),
                          cw_d[:].rearrange("(ct p) k -> p ct k", p=128))
        nc.sync.dma_start(cb[:].rearrange("p (ct o) -> p ct o", ct=2),
                          cb_d[:].rearrange("(ct p) o -> p ct o", p=128))

        # ---- constants: sliding mask, v ones columns, conv pad ----
        # mB[p, c] = 1 if c-512 >= p else 0; diagonal tile with key offset
        # `off` uses slice mB[:, 512-off : 1024-off].
        nc.gpsimd.memset(mB[:], 1.0)
        nc.gpsimd.affine_select(out=mB[:], in_=mB[:], compare_op=ALU.is_ge,
                                fill=0.0, base=-512, pattern=[[1, 1024]],
                                channel_multiplier=-1)
        nc.gpsimd.memset(
            vsb[:].rearrange("p (j c) -> p j c", c=65)[:, :, 64:65], 1.0)
        nc.gpsimd.memset(yT[:, 0:3], 0.0)
        nc.gpsimd.memset(yT[:, YW:YW + 3], 0.0)

        # ---- phase A: projections ----
        with tc.tile_pool(name="ps_a", bufs=3, space="PSUM") as ps_a, \
             tc.tile_pool(name="ps_v", bufs=2, space="PSUM") as ps_v, \
             tc.tile_pool(name="ga", bufs=2) as ga_pool:
            # qT / kT: [128 cols, 512 tok] tiles; m in {0,1} selects 128 q-channels
            for m in range(2):
                for nb in range(S // 512):
                    pk = ps_a.tile([128, 512], F32, tag="ps_a")
                    for kt in range(KT):
                        nc.tensor.matmul(
                            pk[:], wk[:, kt * CL + m * 128: kt * CL + (m + 1) * 128],
                            xm[:, kt * S + nb * 512: kt * S + (nb + 1) * 512],
                            start=(kt == 0), stop=(kt == KT - 1))
                    nc.vector.tensor_copy(kT[:, m * S + nb * 512: m * S + (nb + 1) * 512], pk[:])
                for nb in range(T // 512):
                    pq = ps_a.tile([128, 512], F32, tag="ps_a")
                    for kt in range(KT):
                        nc.tensor.matmul(
                            pq[:], wq[:, kt * CL + m * 128: kt * CL + (m + 1) * 128],
                            xm[:, kt * S + nb * 512: kt * S + (nb + 1) * 512],
                            start=(kt == 0), stop=(kt == KT - 1))
                    nc.vector.tensor_copy(qT[:, m * T + nb * 512: m * T + (nb + 1) * 512], pq[:])
            # gate: [4, 512] psum -> sigmoid(in + b) -> g4 -> flatten rows to part 0
            for nb in range(T // 512):
                pg = ps_v.tile([HL, 512], F32, tag="ps_g")
                for kt in range(KT):
                    nc.tensor.matmul(pg[:], wg[:, kt * HL:(kt + 1) * HL],
                                     xm[:, kt * S + nb * 512: kt * S + (nb + 1) * 512],
                                     start=(kt == 0), stop=(kt == KT - 1))
                nc.scalar.activation(g4[:, nb * 512:(nb + 1) * 512], pg[:],
                                     AF.Sigmoid, bias=gbb[:])
            for h in range(HL):
                nc.sync.dma_start(gfl[:, h * T:(h + 1) * T], g4[h:h + 1, :])
            # v keys-major with interleaved ones columns
            for j in range(NKT):
                pv = ps_v.tile([128, CL], F32, tag="ps_v")
                for kt in range(KT):
                    nc.tensor.matmul(pv[:], xm[:, kt * S + j * 128: kt * S + (j + 1) * 128],
                                     wv[:, kt * CL:(kt + 1) * CL],
                                     start=(kt == 0), stop=(kt == KT - 1))
                dst = vsb[:, j * (HL * 65):(j + 1) * (HL * 65)]
                nc.vector.tensor_copy(
                    dst.rearrange("p (h c) -> p h c", h=HL)[:, :, 0:64],
                    pv[:].rearrange("p (h c) -> p h c", h=HL))

        # ---- phase B: attention ----
        with tc.tile_pool(name="ps_s", bufs=4, space="PSUM") as ps_s, \
             tc.tile_pool(name="ps_acc", bufs=4, space="PSUM") as ps_acc, \
             tc.tile_pool(name="e", bufs=4) as e_pool, \
             tc.tile_pool(name="zr", bufs=2) as zr_pool, \
             tc.tile_pool(name="bc", bufs=2) as bc_pool, \
             tc.tile_pool(name="ct", bufs=2) as ct_pool:
            for m in range(2):            # head pair (rows 0-63 / 64-127)
                for b in range(NQB):      # query block of 512 tokens
                    accs = {}
                    for hh in range(2):
                        accs[hh, "c"] = ps_acc.tile([65, 512], F32, tag="acc", name="acc_c")
                        accs[hh, "m"] = ps_acc.tile([65, 512], F32, tag="acc", name="acc_m")
                    njc = 4 * (b + 1)     # chunk key tiles for this block
                    js = list(range(njc)) + list(range(16, 20))
                    for j in js:
                        is_mem = j >= 16
                        for hh in range(2):
                            h = 2 * m + hh
                            r0 = hh * 64
                            ps = ps_s.tile([128, 512], F32, tag="ps_s")
                            nc.tensor.matmul(
                                ps[:],
                                kT[r0:r0 + 64, m * S + j * 128: m * S + (j + 1) * 128],
                                qT[r0:r0 + 64, m * T + b * 512: m * T + (b + 1) * 512],
                                start=True, stop=True)
                            et = e_pool.tile([128, 512], BF16, tag="e")
                            nc.scalar.activation(et[:], ps[:], AF.Exp)
                            if not is_mem and j >= 4 * b:
                                off = (j - 4 * b) * 128
                                nc.vector.tensor_mul(
                                    et[:], et[:], mB[:, 512 - off: 1024 - off])
                            acc = accs[hh, "m" if is_mem else "c"]
                            jj = js.index(j)
                            first = (jj == 0) if not is_mem else (j == 16)
                            last = (jj == njc - 1) if not is_mem else (j == 19)
                            nc.tensor.matmul(
                                acc[:], vsb[:, j * (HL * 65) + h * 65: j * (HL * 65) + h * 65 + 65],
                                et[:], start=first, stop=last)
                    # combine: y = (yc + g*ym) / Z
                    for hh in range(2):
                        h = 2 * m + hh
                        pc, pm = accs[hh, "c"], accs[hh, "m"]
                        zc = zr_pool.tile([1, 512], F32, tag="zc")
                        nc.vector.tensor_copy(zc[:], pc[64:65, :])
                        z = zr_pool.tile([1, 512], F32, tag="z")
                        nc.vector.tensor_add(z[:], zc[:], pm[64:65, :])
                        r = zr_pool.tile([1, 512], F32, tag="r")
                        nc.vector.reciprocal(r[:], z[:])
                        rb = bc_pool.tile([64, 512], F32, tag="rb")
                        nc.gpsimd.partition_broadcast(rb[:], r[:], channels=64)
                        gB = bc_pool.tile([64, 512], F32, tag="gB")
                        nc.gpsimd.partition_broadcast(
                            gB[:], gfl[:, h * T + b * 512: h * T + (b + 1) * 512],
                            channels=64)
                        t1 = ct_pool.tile([64, 512], F32, tag="t1")
                        nc.vector.tensor_mul(t1[:], pm[0:64, :], gB[:])
                        t2 = ct_pool.tile([64, 512], F32, tag="t2")
                        nc.vector.tensor_add(t2[:], pc[0:64, :], t1[:])
                        r0 = hh * 64
                        nc.vector.tensor_mul(
                            yT[r0:r0 + 64, m * YW + 3 + b * 512: m * YW + 3 + (b + 1) * 512],
                            t2[:], rb[:])

        # ---- phase C: causal depthwise conv + bias + residual ----
        with tc.tile_pool(name="cv", bufs=2) as cv_pool:
            for ct in range(2):
                o = ct * YW
                acc = cv_pool.tile([128, T], F32, tag="cacc")
                nc.vector.tensor_scalar_mul(acc[:], yT[:, o: o + T],
                                            cw[:, ct * KC: ct * KC + 1])
                for kk in range(1, KC):
                    acc2 = cv_pool.tile([128, T], F32, tag="cacc")
                    nc.vector.scalar_tensor_tensor(
                        acc2[:], yT[:, o + kk: o + kk + T],
                        cw[:, ct * KC + kk: ct * KC + kk + 1], acc[:],
                        op0=ALU.mult, op1=ALU.add)
                    acc = acc2
                nc.vector.scalar_tensor_tensor(
                    ytot[:, ct * T:(ct + 1) * T], acc[:], cb[:, ct: ct + 1],
                    yT[:, o + 3: o + 3 + T], op0=ALU.add, op1=ALU.add)

        # ---- phase D: out projection + store ----
        with tc.tile_pool(name="ps_o", bufs=4, space="PSUM") as ps_o, \
             tc.tile_pool(name="ob", bufs=3) as ob_pool:
            for tt in range(T // 128):
                ot = ob_pool.tile([128, C], F32, tag="ob")
                for nh in range(2):
                    po = ps_o.tile([128, 512], F32, tag="ps_o")
                    for ct in range(2):
                        nc.tensor.matmul(
                            po[:], ytot[:, ct * T + tt * 128: ct * T + (tt + 1) * 128],
                            wo[:, ct * C + nh * 512: ct * C + nh * 512 + 512],
                            start=(ct == 0), stop=(ct == 1))
                    nc.vector.tensor_copy(ot[:, nh * 512:(nh + 1) * 512], po[:])
                nc.sync.dma_start(out_d[tt * 128:(tt + 1) * 128, :], ot[:])

    nc.compile()
    return nc


_NC = None


def _get_nc():
    global _NC
    if _NC is None:
        _NC = build_nc()
    return _NC


def prep_in_maps(inputs):
    bf = ml_dtypes.bfloat16
    f32 = np.float32
    x = np.asarray(inputs["x"], f32)
    fm = np.asarray(inputs["forward_memory"], f32)
    rm = np.asarray(inputs["reverse_memory"], f32)
    Wq = np.asarray(inputs["Wq"], f32)
    Wk = np.asarray(inputs["Wk"], f32)
    Wv = np.asarray(inputs["Wv"], f32)
    Wo = np.asarray(inputs["Wo"], f32)
    gW = np.asarray(inputs["gate_W"], f32)
    gb = np.asarray(inputs["gate_b"], f32)
    cwf = np.asarray(inputs["canon_w"], f32)
    cbf = np.asarray(inputs["canon_bias"], f32)
    wg_eff = Wq @ gW  # gate reads q = x@Wq, so fold: x @ (Wq gate_W)
    scale = f32(1.0 / np.sqrt(D))
    in_maps = []
    for core in range(8):
        b, g = divmod(core, 4)
        cs = slice(g * CL, (g + 1) * CL)
        hs = slice(g * HL, (g + 1) * HL)
        xmT = np.ascontiguousarray(np.concatenate([x[b], fm[b], rm[b]], 0).T)
        in_maps.append({
            "xmT": xmT.astype(bf),
            "wq": np.ascontiguousarray(Wq[:, cs] * scale).astype(bf),
            "wk": np.ascontiguousarray(Wk[:, cs]).astype(bf),
            "wv": np.ascontiguousarray(Wv[:, cs]).astype(bf),
            "wo": np.ascontiguousarray(Wo[cs, :]).astype(bf),
            "wg": np.ascontiguousarray(wg_eff[:, hs]).astype(bf),
            "gb": np.ascontiguousarray(gb[hs]).reshape(HL, 1).astype(f32),
            "cw": np.ascontiguousarray(cwf[cs, 0, :]).astype(f32),
            "cb": np.ascontiguousarray(cbf[cs]).reshape(CL, 1).astype(f32),
        })
    return in_maps


def combine_results(parts):
    out = np.empty((B, T, C), np.float32)
    for b in range(B):
        out[b] = parts[4 * b] + parts[4 * b + 1] + parts[4 * b + 2] + parts[4 * b + 3]
    return out


def kernel(**inputs):
    nc = _get_nc()
    in_maps = prep_in_maps(inputs)
    res = run_bass_kernel_spmd(nc, in_maps, core_ids=list(range(8)))
    return combine_results([res.results[i]["out"] for i in range(8)])


# revision 8
# speedup vs baseline: 1.1186x; 1.1186x over previous
"""CascadeMemoryAttention Trainium2 kernel.

Distribution over 8 NeuronCores: core = b*4 + g handles batch b (of 2) and
head group g (4 heads, 256 q/k/v channels). Per core, everything is computed
in a transposed ("channels-major") layout so that no on-device transposes are
needed anywhere:

  qT  [256,2048] = Wq_loc^T x^T          (host supplies x^T; Wq pre-scaled 1/sqrt(d))
  kT  [256,2560] = Wk_loc^T [x;fm;rm]^T
  v   [2560,256] keys-major (natural matmul output orientation)
  sT  [keys,tok] = kT_h^T qT_h per head  -> exp -> eT (bf16)
  yT  [64,tok]   = [v_h | 1]^T eT        (ones column gives Z = sum(e) for free)
  y   = (y_chunk + g * y_mem) / Z        (gate fused via broadcast rows)
  conv/residual in channels-major, then out_partial = y_tot^T Wo_loc and the
  host sums the 4 partials per batch.

Softmax max-subtraction is skipped: scores are ~N(0, 0.17), |s| < ~6, exp is
safe in fp32/bf16 and softmax is shift-invariant, so results are identical.
"""

import sys

sys.path.insert(0, "/opt/trn_rl_repo")

from contextlib import ExitStack

import numpy as np
import ml_dtypes

import concourse.bass as bass  # noqa: F401  (bass types reachable via bacc)
import concourse.mybir as mybir
import concourse.tile as tile
from concourse import bacc
from concourse.bass_utils import run_bass_kernel_spmd

F32 = mybir.dt.float32
BF16 = mybir.dt.bfloat16
AF = mybir.ActivationFunctionType
ALU = mybir.AluOpType

# Problem dims (hardcoded per the task contract).
B, T, C, H, M = 2, 2048, 1024, 16, 256
D = 64          # head dim
HL = 4          # heads per core
CL = HL * D     # 256 local channels
S = T + 2 * M   # 2560 keys
KT = 8          # feature k-tiles of 128
NKT = S // 128  # 20 key tiles (16 chunk + 4 memory)
NQB = T // 512  # 4 query blocks of 512 tokens
KC = 4          # canon conv taps
YW = T + 3      # yT row width incl. 3-col zero pad for causal conv


def build_nc():
    nc = bacc.Bacc("TRN2", target_bir_lowering=False, debug=False)

    xmT_d = nc.dram_tensor("xmT", [C, S], BF16, kind="ExternalInput")
    wq_d = nc.dram_tensor("wq", [C, CL], BF16, kind="ExternalInput")
    wk_d = nc.dram_tensor("wk", [C, CL], BF16, kind="ExternalInput")
    wv_d = nc.dram_tensor("wv", [C, CL], BF16, kind="ExternalInput")
    wo_d = nc.dram_tensor("wo", [CL, C], BF16, kind="ExternalInput")
    wg_d = nc.dram_tensor("wg", [C, HL], BF16, kind="ExternalInput")
    gb_d = nc.dram_tensor("gb", [HL, 1], F32, kind="ExternalInput")
    cw_d = nc.dram_tensor("cw", [CL, KC], F32, kind="ExternalInput")
    cb_d = nc.dram_tensor("cb", [CL, 1], F32, kind="ExternalInput")
    out_d = nc.dram_tensor("out", [T, C], F32, kind="ExternalOutput")

    with ExitStack() as ctx:
        tc = ctx.enter_context(tile.TileContext(nc))
        pp = ctx.enter_context(tc.tile_pool(name="persist", bufs=1))

        # ---- persistent SBUF tensors ----
        xm = pp.tile([128, KT * S], BF16, tag="xm")        # feature k-tile kt at [kt*S:]
        wq = pp.tile([128, KT * CL], BF16, tag="wq")
        wk = pp.tile([128, KT * CL], BF16, tag="wk")
        wv = pp.tile([128, KT * CL], BF16, tag="wv")
        wg = pp.tile([128, KT * HL], BF16, tag="wg")
        wo = pp.tile([128, 2 * C], BF16, tag="wo")         # ctile ct at [ct*C:]
        gbb = pp.tile([HL, 1], F32, tag="gbb")
        cw = pp.tile([128, 2 * KC], F32, tag="cw")
        cb = pp.tile([128, 2], F32, tag="cb")
        qT = pp.tile([128, 2 * T], BF16, tag="qT")         # m-tile at [m*T:]
        kT = pp.tile([128, 2 * S], BF16, tag="kT")
        vsb = pp.tile([128, NKT * (HL * 65)], BF16, tag="v")  # per key tile: 4x[64 v | 1]
        g4 = pp.tile([HL, T], F32, tag="g4")               # sigmoid gates, partition=head
        gfl = pp.tile([1, HL * T], F32, tag="gfl")         # gates flattened to partition 0
        mB = pp.tile([128, 1024], BF16, tag="mB")          # sliding causal mask
        yT = pp.tile([128, 2 * YW], F32, tag="yT")         # attention out, ch-major
        ytot = pp.tile([128, 2 * T], BF16, tag="ytot")     # after conv+residual

        # ---- input DMAs (weights first; xm split per k-tile for overlap) ----
        nc.sync.dma_start(wq[:].rearrange("p (kt c) -> p kt c", kt=KT),
                          wq_d[:].rearrange("(kt p) c -> p kt c", p=128))
        nc.sync.dma_start(wk[:].rearrange("p (kt c) -> p kt c", kt=KT),
                          wk_d[:].rearrange("(kt p) c -> p kt c", p=128))
        nc.sync.dma_start(wv[:].rearrange("p (kt c) -> p kt c", kt=KT),
                          wv_d[:].rearrange("(kt p) c -> p kt c", p=128))
        nc.sync.dma_start(wg[:].rearrange("p (kt h) -> p kt h", kt=KT),
                          wg_d[:].rearrange("(kt p) h -> p kt h", p=128))
        nc.sync.dma_start(wo[:].rearrange("p (ct n) -> p ct n", ct=2),
                          wo_d[:].rearrange("(ct p) n -> p ct n", p=128))
        nc.sync.dma_start(gbb[:], gb_d[:])
        nc.sync.dma_start(cw[:].rearrange("p (ct k) -> p ct k", ct=2),
                          cw_d[:].rearrange("(ct p) k -> p ct k", p=128))
        nc.sync.dma_start(cb[:].rearrange("p (ct o) -> p ct o", ct=2),
                          cb_d[:].rearrange("(ct p) o -> p ct o", p=128))
        for kt in range(KT):
            nc.sync.dma_start(xm[:, kt * S:(kt + 1) * S],
                              xmT_d[kt * 128:(kt + 1) * 128, :])

        # ---- constants: sliding mask, v ones columns, conv pad ----
        # mB[p, c] = 1 if c-512 >= p else 0; diagonal tile with key offset
        # `off` uses slice mB[:, 512-off : 1024-off].
        nc.gpsimd.memset(mB[:], 1.0)
        nc.gpsimd.affine_select(out=mB[:], in_=mB[:], compare_op=ALU.is_ge,
                                fill=0.0, base=-512, pattern=[[1, 1024]],
                                channel_multiplier=-1)
        nc.gpsimd.memset(
            vsb[:].rearrange("p (j c) -> p j c", c=65)[:, :, 64:65], 1.0)
        nc.gpsimd.memset(yT[:, 0:3], 0.0)
        nc.gpsimd.memset(yT[:, YW:YW + 3], 0.0)

        # ---- phase A: projections ----
        with tc.tile_pool(name="ps_a", bufs=3, space="PSUM") as ps_a, \
             tc.tile_pool(name="ps_v", bufs=2, space="PSUM") as ps_v, \
             tc.tile_pool(name="ga", bufs=2) as ga_pool:
            # qT / kT: [128 cols, 512 tok] tiles; m in {0,1} selects 128 q-channels
            for m in range(2):
                for nb in range(S // 512):
                    pk = ps_a.tile([128, 512], F32, tag="ps_a")
                    for kt in range(KT):
                        nc.tensor.matmul(
                            pk[:], wk[:, kt * CL + m * 128: kt * CL + (m + 1) * 128],
                            xm[:, kt * S + nb * 512: kt * S + (nb + 1) * 512],
                            start=(kt == 0), stop=(kt == KT - 1))
                    nc.vector.tensor_copy(kT[:, m * S + nb * 512: m * S + (nb + 1) * 512], pk[:])
                for nb in range(T // 512):
                    pq = ps_a.tile([128, 512], F32, tag="ps_a")
                    for kt in range(KT):
                        nc.tensor.matmul(
                            pq[:], wq[:, kt * CL + m * 128: kt * CL + (m + 1) * 128],
                            xm[:, kt * S + nb * 512: kt * S + (nb + 1) * 512],
                            start=(kt == 0), stop=(kt == KT - 1))
                    nc.vector.tensor_copy(qT[:, m * T + nb * 512: m * T + (nb + 1) * 512], pq[:])
            # gate: [4, 512] psum -> sigmoid(in + b) -> g4 -> flatten rows to part 0
            for nb in range(T // 512):
                pg = ps_v.tile([HL, 512], F32, tag="ps_g")
                for kt in range(KT):
                    nc.tensor.matmul(pg[:], wg[:, kt * HL:(kt + 1) * HL],
                                     xm[:, kt * S + nb * 512: kt * S + (nb + 1) * 512],
                                     start=(kt == 0), stop=(kt == KT - 1))
                nc.scalar.activation(g4[:, nb * 512:(nb + 1) * 512], pg[:],
                                     AF.Sigmoid, bias=gbb[:])
            for h in range(HL):
                nc.sync.dma_start(gfl[:, h * T:(h + 1) * T], g4[h:h + 1, :])
            # v keys-major with interleaved ones columns
            for j in range(NKT):
                pv = ps_v.tile([128, CL], F32, tag="ps_v")
                for kt in range(KT):
                    nc.tensor.matmul(pv[:], xm[:, kt * S + j * 128: kt * S + (j + 1) * 128],
                                     wv[:, kt * CL:(kt + 1) * CL],
                                     start=(kt == 0), stop=(kt == KT - 1))
                dst = vsb[:, j * (HL * 65):(j + 1) * (HL * 65)]
                nc.vector.tensor_copy(
                    dst.rearrange("p (h c) -> p h c", h=HL)[:, :, 0:64],
                    pv[:].rearrange("p (h c) -> p h c", h=HL))

        # ---- phases B/C/D fused per 512-token block: attention -> conv -> out ----
        with tc.tile_pool(name="ps_s", bufs=3, space="PSUM") as ps_s, \
             tc.tile_pool(name="ps_acc", bufs=4, space="PSUM") as ps_acc, \
             tc.tile_pool(name="ps_o", bufs=1, space="PSUM") as ps_o, \
             tc.tile_pool(name="e", bufs=4) as e_pool, \
             tc.tile_pool(name="zr", bufs=2) as zr_pool, \
             tc.tile_pool(name="bc", bufs=2) as bc_pool, \
             tc.tile_pool(name="ct", bufs=2) as ct_pool, \
             tc.tile_pool(name="cv", bufs=2) as cv_pool, \
             tc.tile_pool(name="ob", bufs=3) as ob_pool:
            for b in range(NQB):          # query block of 512 tokens
                for m in range(2):        # head pair (rows 0-63 / 64-127)
                    accs = {}
                    for hh in range(2):
                        accs[hh, "c"] = ps_acc.tile([65, 512], F32, tag="acc", name="acc_c")
                        accs[hh, "m"] = ps_acc.tile([65, 512], F32, tag="acc", name="acc_m")
                    njc = 4 * (b + 1)     # chunk key tiles for this block
                    js = list(range(njc)) + list(range(16, 20))
                    for j in js:
                        is_mem = j >= 16
                        pss = []
                        # the two heads' K=64 score matmuls sit on disjoint
                        # PE row groups (0-63 / 64-127) and run concurrently
                        for hh in range(2):
                            r0 = hh * 64
                            ps = ps_s.tile([128, 512], F32, tag="ps_s")
                            nc.tensor.matmul(
                                ps[:],
                                kT[r0:r0 + 64, m * S + j * 128: m * S + (j + 1) * 128],
                                qT[r0:r0 + 64, m * T + b * 512: m * T + (b + 1) * 512],
                                start=True, stop=True, tile_position=(r0, 0))
                            pss.append(ps)
                        jj = js.index(j)
                        first = (jj == 0) if not is_mem else (j == 16)
                        last = (jj == njc - 1) if not is_mem else (j == 19)
                        for hh in range(2):
                            h = 2 * m + hh
                            et = e_pool.tile([128, 512], BF16, tag="e")
                            nc.scalar.activation(et[:], pss[hh][:], AF.Exp)
                            if not is_mem and j >= 4 * b:
                                off = (j - 4 * b) * 128
                                nc.vector.tensor_mul(
                                    et[:], et[:], mB[:, 512 - off: 1024 - off])
                            acc = accs[hh, "m" if is_mem else "c"]
                            nc.tensor.matmul(
                                acc[:], vsb[:, j * (HL * 65) + h * 65: j * (HL * 65) + h * 65 + 65],
                                et[:], start=first, stop=last)
                    # combine: y = (yc + g*ym) / Z
                    for hh in range(2):
                        h = 2 * m + hh
                        pc, pm = accs[hh, "c"], accs[hh, "m"]
                        zc = zr_pool.tile([1, 512], F32, tag="zc")
                        nc.scalar.copy(zc[:], pc[64:65, :])
                        z = zr_pool.tile([1, 512], F32, tag="z")
                        nc.vector.tensor_add(z[:], zc[:], pm[64:65, :])
                        zb = bc_pool.tile([64, 512], F32, tag="zb")
                        nc.gpsimd.partition_broadcast(zb[:], z[:], channels=64)
                        rb = bc_pool.tile([64, 512], F32, tag="rb")
                        nc.vector.reciprocal(rb[:], zb[:])
                        gB = bc_pool.tile([64, 512], F32, tag="gB")
                        nc.gpsimd.partition_broadcast(
                            gB[:], gfl[:, h * T + b * 512: h * T + (b + 1) * 512],
                            channels=64)
                        t1 = ct_pool.tile([64, 512], F32, tag="t1")
                        nc.vector.tensor_mul(t1[:], pm[0:64, :], gB[:])
                        t2 = ct_pool.tile([64, 512], F32, tag="t2")
                        nc.vector.tensor_add(t2[:], pc[0:64, :], t1[:])
                        r0 = hh * 64
                        nc.vector.tensor_mul(
                            yT[r0:r0 + 64, m * YW + 3 + b * 512: m * YW + 3 + (b + 1) * 512],
                            t2[:], rb[:])
                # conv + bias + residual for this block (both channel tiles)
                for ctile in range(2):
                    o = ctile * YW + b * 512
                    acc = cv_pool.tile([128, 512], F32, tag="cacc")
                    nc.vector.tensor_scalar_mul(acc[:], yT[:, o: o + 512],
                                                cw[:, ctile * KC: ctile * KC + 1])
                    for kk in range(1, KC):
                        acc2 = cv_pool.tile([128, 512], F32, tag="cacc")
                        nc.vector.scalar_tensor_tensor(
                            acc2[:], yT[:, o + kk: o + kk + 512],
                            cw[:, ctile * KC + kk: ctile * KC + kk + 1], acc[:],
                            op0=ALU.mult, op1=ALU.add)
                        acc = acc2
                    nc.vector.scalar_tensor_tensor(
                        ytot[:, ctile * T + b * 512: ctile * T + (b + 1) * 512],
                        acc[:], cb[:, ctile: ctile + 1],
                        yT[:, o + 3: o + 3 + 512], op0=ALU.add, op1=ALU.add)
                # out projection for this block's 4 token tiles
                for tt in range(4 * b, 4 * b + 4):
                    ot = ob_pool.tile([128, C], F32, tag="ob")
                    for nh in range(2):
                        po = ps_o.tile([128, 512], F32, tag="ps_o")
                        for ctile in range(2):
                            nc.tensor.matmul(
                                po[:], ytot[:, ctile * T + tt * 128: ctile * T + (tt + 1) * 128],
                                wo[:, ctile * C + nh * 512: ctile * C + nh * 512 + 512],
                                start=(ctile == 0), stop=(ctile == 1))
                        nc.vector.tensor_copy(ot[:, nh * 512:(nh + 1) * 512], po[:])
                    nc.sync.dma_start(out_d[tt * 128:(tt + 1) * 128, :], ot[:])

    nc.compile()
    return nc


_NC = None


def _get_nc():
    global _NC
    if _NC is None:
        _NC = build_nc()
    return _NC


def prep_in_maps(inputs):
    bf = ml_dtypes.bfloat16
    f32 = np.float32
    x = np.asarray(inputs["x"], f32)
    fm = np.asarray(inputs["forward_memory"], f32)
    rm = np.asarray(inputs["reverse_memory"], f32)
    Wq = np.asarray(inputs["Wq"], f32)
    Wk = np.asarray(inputs["Wk"], f32)
    Wv = np.asarray(inputs["Wv"], f32)
    Wo = np.asarray(inputs["Wo"], f32)
    gW = np.asarray(inputs["gate_W"], f32)
    gb = np.asarray(inputs["gate_b"], f32)
    cwf = np.asarray(inputs["canon_w"], f32)
    cbf = np.asarray(inputs["canon_bias"], f32)
    wg_eff = Wq @ gW  # gate reads q = x@Wq, so fold: x @ (Wq gate_W)
    scale = f32(1.0 / np.sqrt(D))
    in_maps = []
    for core in range(8):
        b, g = divmod(core, 4)
        cs = slice(g * CL, (g + 1) * CL)
        hs = slice(g * HL, (g + 1) * HL)
        xmT = np.ascontiguousarray(np.concatenate([x[b], fm[b], rm[b]], 0).T)
        in_maps.append({
            "xmT": xmT.astype(bf),
            "wq": np.ascontiguousarray(Wq[:, cs] * scale).astype(bf),
            "wk": np.ascontiguousarray(Wk[:, cs]).astype(bf),
            "wv": np.ascontiguousarray(Wv[:, cs]).astype(bf),
            "wo": np.ascontiguousarray(Wo[cs, :]).astype(bf),
            "wg": np.ascontiguousarray(wg_eff[:, hs]).astype(bf),
            "gb": np.ascontiguousarray(gb[hs]).reshape(HL, 1).astype(f32),
            "cw": np.ascontiguousarray(cwf[cs, 0, :]).astype(f32),
            "cb": np.ascontiguousarray(cbf[cs]).reshape(CL, 1).astype(f32),
        })
    return in_maps


def combine_results(parts):
    out = np.empty((B, T, C), np.float32)
    for b in range(B):
        out[b] = parts[4 * b] + parts[4 * b + 1] + parts[4 * b + 2] + parts[4 * b + 3]
    return out


def kernel(**inputs):
    nc = _get_nc()
    in_maps = prep_in_maps(inputs)
    res = run_bass_kernel_spmd(nc, in_maps, core_ids=list(range(8)))
    return combine_results([res.results[i]["out"] for i in range(8)])


# revision 14
# speedup vs baseline: 1.1816x; 1.0563x over previous
"""CascadeMemoryAttention Trainium2 kernel.

Distribution over 8 NeuronCores: core = b*4 + g handles batch b (of 2) and
head group g (4 heads, 256 q/k/v channels). Per core, everything is computed
in a transposed ("channels-major") layout so that no on-device transposes are
needed anywhere:

  qT  [256,2048] = Wq_loc^T x^T          (host supplies x^T; Wq pre-scaled 1/sqrt(d))
  kT  [256,2560] = Wk_loc^T [x;fm;rm]^T
  v   [2560,256] keys-major (natural matmul output orientation)
  sT  [keys,tok] = kT_h^T qT_h per head  -> exp -> eT (bf16)
  yT  [64,tok]   = [v_h | 1]^T eT        (ones column gives Z = sum(e) for free)
  y   = (y_chunk + g * y_mem) / Z        (gate fused via broadcast rows)
  conv/residual in channels-major, then out_partial = y_tot^T Wo_loc and the
  host sums the 4 partials per batch.

Softmax max-subtraction is skipped: scores are ~N(0, 0.17), |s| < ~6, exp is
safe in fp32/bf16 and softmax is shift-invariant, so results are identical.
"""

import sys

sys.path.insert(0, "/opt/trn_rl_repo")

from contextlib import ExitStack

import numpy as np
import ml_dtypes

import concourse.bass as bass  # noqa: F401  (bass types reachable via bacc)
import concourse.mybir as mybir
import concourse.tile as tile
from concourse import bacc
from concourse.bass_utils import run_bass_kernel_spmd

F32 = mybir.dt.float32
BF16 = mybir.dt.bfloat16
AF = mybir.ActivationFunctionType
ALU = mybir.AluOpType

# Problem dims (hardcoded per the task contract).
B, T, C, H, M = 2, 2048, 1024, 16, 256
D = 64          # head dim
HL = 4          # heads per core
CL = HL * D     # 256 local channels
S = T + 2 * M   # 2560 keys
KT = 8          # feature k-tiles of 128
NKT = S // 128  # 20 key tiles (16 chunk + 4 memory)
NQB = T // 512  # 4 query blocks of 512 tokens
KC = 4          # canon conv taps
YW = T + 3      # yT row width incl. 3-col zero pad for causal conv


def build_nc():
    nc = bacc.Bacc("TRN2", target_bir_lowering=False, debug=False)

    xmT_d = nc.dram_tensor("xmT", [C, S], BF16, kind="ExternalInput")
    wq_d = nc.dram_tensor("wq", [C, CL], BF16, kind="ExternalInput")
    wk_d = nc.dram_tensor("wk", [C, CL], BF16, kind="ExternalInput")
    wv_d = nc.dram_tensor("wv", [C, CL], BF16, kind="ExternalInput")
    wo_d = nc.dram_tensor("wo", [CL, C], BF16, kind="ExternalInput")
    wg_d = nc.dram_tensor("wg", [C, HL], BF16, kind="ExternalInput")
    gb_d = nc.dram_tensor("gb", [HL, 1], F32, kind="ExternalInput")
    cw_d = nc.dram_tensor("cw", [CL, KC], F32, kind="ExternalInput")
    cb_d = nc.dram_tensor("cb", [CL, 1], F32, kind="ExternalInput")
    out_d = nc.dram_tensor("out", [T, C], F32, kind="ExternalOutput")

    with ExitStack() as ctx:
        tc = ctx.enter_context(tile.TileContext(nc))
        pp = ctx.enter_context(tc.tile_pool(name="persist", bufs=1))

        # ---- persistent SBUF tensors ----
        xm = pp.tile([128, KT * S], BF16, tag="xm")        # feature k-tile kt at [kt*S:]
        wq = pp.tile([128, KT * CL], BF16, tag="wq")
        wk = pp.tile([128, KT * CL], BF16, tag="wk")
        wv = pp.tile([128, KT * CL], BF16, tag="wv")
        wg = pp.tile([128, KT * HL], BF16, tag="wg")
        wo = pp.tile([128, 2 * C], BF16, tag="wo")         # ctile ct at [ct*C:]
        gbb = pp.tile([HL, 1], F32, tag="gbb")
        cw = pp.tile([128, 2 * KC], F32, tag="cw")
        cb = pp.tile([128, 2], F32, tag="cb")
        qT = pp.tile([128, 2 * T], BF16, tag="qT")         # m-tile at [m*T:]
        kT = pp.tile([128, 2 * S], BF16, tag="kT")
        vsb = pp.tile([128, NKT * CL], BF16, tag="v")      # keys-major v, key tile j at [j*CL:]
        ones1 = pp.tile([128, 1], BF16, tag="ones1")       # Z-row matmul weights
        g4 = pp.tile([HL, T], F32, tag="g4")               # sigmoid gates, partition=head
        gfl = pp.tile([1, HL * T], F32, tag="gfl")         # gates flattened to partition 0
        mB = pp.tile([128, 1024], BF16, tag="mB")          # sliding causal mask
        yT = pp.tile([128, 2 * YW], F32, tag="yT")         # attention out, ch-major
        ytot = pp.tile([128, 2 * T], BF16, tag="ytot")     # after conv+residual

        # ---- input DMAs (weights first; xm split per k-tile for overlap) ----
        nc.sync.dma_start(wq[:].rearrange("p (kt c) -> p kt c", kt=KT),
                          wq_d[:].rearrange("(kt p) c -> p kt c", p=128))
        nc.sync.dma_start(wk[:].rearrange("p (kt c) -> p kt c", kt=KT),
                          wk_d[:].rearrange("(kt p) c -> p kt c", p=128))
        nc.sync.dma_start(wv[:].rearrange("p (kt c) -> p kt c", kt=KT),
                          wv_d[:].rearrange("(kt p) c -> p kt c", p=128))
        nc.sync.dma_start(wg[:].rearrange("p (kt h) -> p kt h", kt=KT),
                          wg_d[:].rearrange("(kt p) h -> p kt h", p=128))
        nc.sync.dma_start(wo[:].rearrange("p (ct n) -> p ct n", ct=2),
                          wo_d[:].rearrange("(ct p) n -> p ct n", p=128))
        nc.sync.dma_start(gbb[:], gb_d[:])
        nc.sync.dma_start(cw[:].rearrange("p (ct k) -> p ct k", ct=2),
                          cw_d[:].rearrange("(ct p) k -> p ct k", p=128))
        nc.sync.dma_start(cb[:].rearrange("p (ct o) -> p ct o", ct=2),
                          cb_d[:].rearrange("(ct p) o -> p ct o", p=128))
        for kt in range(KT):
            nc.sync.dma_start(xm[:, kt * S:(kt + 1) * S],
                              xmT_d[kt * 128:(kt + 1) * 128, :])

        # ---- constants: sliding mask, v ones columns, conv pad ----
        # mB[p, c] = 1 if c-512 >= p else 0; diagonal tile with key offset
        # `off` uses slice mB[:, 512-off : 1024-off].
        nc.gpsimd.memset(mB[:], 1.0)
        nc.gpsimd.affine_select(out=mB[:], in_=mB[:], compare_op=ALU.is_ge,
                                fill=0.0, base=-512, pattern=[[1, 1024]],
                                channel_multiplier=-1)
        nc.gpsimd.memset(ones1[:], 1.0)
        nc.gpsimd.memset(yT[:, 0:3], 0.0)
        nc.gpsimd.memset(yT[:, YW:YW + 3], 0.0)

        # ---- phase A: projections ----
        with tc.tile_pool(name="ps_a", bufs=3, space="PSUM") as ps_a, \
             tc.tile_pool(name="ps_v", bufs=2, space="PSUM") as ps_v, \
             tc.tile_pool(name="ga", bufs=2) as ga_pool:
            # qT / kT: [128 cols, 512 tok] tiles; m in {0,1} selects 128 q-channels
            for m in range(2):
                for nb in range(S // 512):
                    pk = ps_a.tile([128, 512], F32, tag="ps_a")
                    for kt in range(KT):
                        nc.tensor.matmul(
                            pk[:], wk[:, kt * CL + m * 128: kt * CL + (m + 1) * 128],
                            xm[:, kt * S + nb * 512: kt * S + (nb + 1) * 512],
                            start=(kt == 0), stop=(kt == KT - 1))
                    nc.vector.tensor_copy(kT[:, m * S + nb * 512: m * S + (nb + 1) * 512], pk[:])
                for nb in range(T // 512):
                    pq = ps_a.tile([128, 512], F32, tag="ps_a")
                    for kt in range(KT):
                        nc.tensor.matmul(
                            pq[:], wq[:, kt * CL + m * 128: kt * CL + (m + 1) * 128],
                            xm[:, kt * S + nb * 512: kt * S + (nb + 1) * 512],
                            start=(kt == 0), stop=(kt == KT - 1))
                    nc.vector.tensor_copy(qT[:, m * T + nb * 512: m * T + (nb + 1) * 512], pq[:])
            # gate: [4, 512] psum -> sigmoid(in + b) -> g4 -> flatten rows to part 0
            for nb in range(T // 512):
                pg = ps_v.tile([HL, 512], F32, tag="ps_g")
                for kt in range(KT):
                    nc.tensor.matmul(pg[:], wg[:, kt * HL:(kt + 1) * HL],
                                     xm[:, kt * S + nb * 512: kt * S + (nb + 1) * 512],
                                     start=(kt == 0), stop=(kt == KT - 1))
                nc.scalar.activation(g4[:, nb * 512:(nb + 1) * 512], pg[:],
                                     AF.Sigmoid, bias=gbb[:])
            for h in range(HL):
                nc.sync.dma_start(gfl[:, h * T:(h + 1) * T], g4[h:h + 1, :])
            # v keys-major with interleaved ones columns
            for j in range(NKT):
                pv = ps_v.tile([128, CL], F32, tag="ps_v")
                for kt in range(KT):
                    nc.tensor.matmul(pv[:], xm[:, kt * S + j * 128: kt * S + (j + 1) * 128],
                                     wv[:, kt * CL:(kt + 1) * CL],
                                     start=(kt == 0), stop=(kt == KT - 1))
                nc.vector.tensor_copy(vsb[:, j * CL:(j + 1) * CL], pv[:])

        # ---- phases B/C/D fused per 512-token block: attention -> conv -> out ----
        with tc.tile_pool(name="ps_s", bufs=2, space="PSUM") as ps_s, \
             tc.tile_pool(name="ps_acc", bufs=4, space="PSUM") as ps_acc, \
             tc.tile_pool(name="e", bufs=4) as e_pool, \
             tc.tile_pool(name="zr", bufs=2) as zr_pool, \
             tc.tile_pool(name="bc", bufs=2) as bc_pool, \
             tc.tile_pool(name="ct", bufs=2) as ct_pool, \
             tc.tile_pool(name="cv", bufs=2) as cv_pool, \
             tc.tile_pool(name="ob", bufs=3) as ob_pool:
            for b in range(NQB):          # query block of 512 tokens
                for m in range(2):        # head pair (rows 0-63 / 64-127)
                    # accC/accM: [h0 rows 0-63 | h1 rows 64-127] via column-
                    # tiled M=64 matmul pairs. pz holds 4 Z accumulators at
                    # rows 32*(2*parity + hh), accumulated over ALL key tiles.
                    accC = ps_acc.tile([128, 512], F32, tag="acc", name="accC")
                    accM = ps_acc.tile([128, 512], F32, tag="acc", name="accM")
                    pz = ps_acc.tile([128, 512], F32, tag="acc", name="pz")
                    njc = 4 * (b + 1)     # chunk key tiles for this block
                    js = list(range(njc)) + list(range(16, 20))
                    np_pairs = len(js) // 2
                    for pi in range(np_pairs):
                        j0, j1 = js[2 * pi], js[2 * pi + 1]
                        # scores: the two heads' K=64 matmuls sit on disjoint
                        # PE row groups (0-63 / 64-127) and run concurrently
                        pss = [ps_s.tile([128, 1024], F32, tag="ps_s", name="ps_s")
                               for _ in range(2)]
                        for idx, j in ((0, j0), (1, j1)):
                            for hh in range(2):
                                r0 = hh * 64
                                nc.tensor.matmul(
                                    pss[hh][:, idx * 512:(idx + 1) * 512],
                                    kT[r0:r0 + 64, m * S + j * 128: m * S + (j + 1) * 128],
                                    qT[r0:r0 + 64, m * T + b * 512: m * T + (b + 1) * 512],
                                    start=True, stop=True, tile_position=(r0, 0))
                        ets = []
                        for hh in range(2):
                            et = e_pool.tile([128, 1024], BF16, tag="e", name="et")
                            nc.scalar.activation(et[:], pss[hh][:], AF.Exp)
                            for idx, j in ((0, j0), (1, j1)):
                                if j < 16 and j >= 4 * b:
                                    off = (j - 4 * b) * 128
                                    nc.vector.tensor_mul(
                                        et[:, idx * 512:(idx + 1) * 512],
                                        et[:, idx * 512:(idx + 1) * 512],
                                        mB[:, 512 - off: 1024 - off])
                            ets.append(et)
                        # PV: column-packed M=64 pairs (h0 cols 0-63, h1 64-127)
                        for idx, j in ((0, j0), (1, j1)):
                            is_mem = j >= 16
                            acc = accM if is_mem else accC
                            first = (j == 0) if not is_mem else (j == 16)
                            last = (j == njc - 1) if not is_mem else (j == 19)
                            for hh in range(2):
                                h = 2 * m + hh
                                c0 = hh * 64
                                nc.tensor.matmul(
                                    acc[c0:c0 + 64, :],
                                    vsb[:, j * CL + h * D: j * CL + h * D + D],
                                    ets[hh][:, idx * 512:(idx + 1) * 512],
                                    start=first, stop=last,
                                    tile_position=(0, c0), skip_group_check=True)
                        # Z rows: 4 concurrent M=1 matmuls on distinct col groups
                        for idx, j in ((0, j0), (1, j1)):
                            for hh in range(2):
                                row = 64 * idx + 32 * hh
                                nc.tensor.matmul(
                                    pz[row:row + 1, :], ones1[:, 0:1],
                                    ets[hh][:, idx * 512:(idx + 1) * 512],
                                    start=(pi == 0), stop=(pi == np_pairs - 1),
                                    tile_position=(0, row), skip_group_check=True)
                    # combine: y = (yc + g*ym) / Z
                    for hh in range(2):
                        h = 2 * m + hh
                        r0 = hh * 64
                        zc = zr_pool.tile([1, 512], F32, tag="zc")
                        nc.scalar.copy(zc[:], pz[32 * hh: 32 * hh + 1, :])
                        z = zr_pool.tile([1, 512], F32, tag="z")
                        nc.vector.tensor_add(z[:], zc[:], pz[64 + 32 * hh: 65 + 32 * hh, :])
                        zb = bc_pool.tile([64, 512], F32, tag="zb")
                        nc.gpsimd.partition_broadcast(zb[:], z[:], channels=64)
                        rb = bc_pool.tile([64, 512], F32, tag="rb")
                        nc.vector.reciprocal(rb[:], zb[:])
                        gB = bc_pool.tile([64, 512], F32, tag="gB")
                        nc.gpsimd.partition_broadcast(
                            gB[:], gfl[:, h * T + b * 512: h * T + (b + 1) * 512],
                            channels=64)
                        t1 = ct_pool.tile([64, 512], F32, tag="t1")
                        nc.vector.tensor_mul(t1[:], accM[r0:r0 + 64, :], gB[:])
                        t2 = ct_pool.tile([64, 512], F32, tag="t2")
                        nc.vector.tensor_add(t2[:], accC[r0:r0 + 64, :], t1[:])
                        nc.vector.tensor_mul(
                            yT[r0:r0 + 64, m * YW + 3 + b * 512: m * YW + 3 + (b + 1) * 512],
                            t2[:], rb[:])
                # conv + bias + residual for this block (both channel tiles)
                for ctile in range(2):
                    o = ctile * YW + b * 512
                    acc = cv_pool.tile([128, 512], F32, tag="cacc")
                    nc.vector.tensor_scalar_mul(acc[:], yT[:, o: o + 512],
                                                cw[:, ctile * KC: ctile * KC + 1])
                    for kk in range(1, KC):
                        acc2 = cv_pool.tile([128, 512], F32, tag="cacc")
                        nc.vector.scalar_tensor_tensor(
                            acc2[:], yT[:, o + kk: o + kk + 512],
                            cw[:, ctile * KC + kk: ctile * KC + kk + 1], acc[:],
                            op0=ALU.mult, op1=ALU.add)
                        acc = acc2
                    nc.vector.scalar_tensor_tensor(
                        ytot[:, ctile * T + b * 512: ctile * T + (b + 1) * 512],
                        acc[:], cb[:, ctile: ctile + 1],
                        yT[:, o + 3: o + 3 + 512], op0=ALU.add, op1=ALU.add)
                # out projection for this block's 4 token tiles
                for tt in range(4 * b, 4 * b + 4):
                    ot = ob_pool.tile([128, C], F32, tag="ob")
                    for nh in range(2):
                        po = ps_acc.tile([128, 512], F32, tag="acc", name="po")
                        for ctile in range(2):
                            nc.tensor.matmul(
                                po[:], ytot[:, ctile * T + tt * 128: ctile * T + (tt + 1) * 128],
                                wo[:, ctile * C + nh * 512: ctile * C + nh * 512 + 512],
                                start=(ctile == 0), stop=(ctile == 1))
                        nc.vector.tensor_copy(ot[:, nh * 512:(nh + 1) * 512], po[:])
                    nc.sync.dma_start(out_d[tt * 128:(tt + 1) * 128, :], ot[:])

    nc.compile()
    return nc


_NC = None


def _get_nc():
    global _NC
    if _NC is None:
        _NC = build_nc()
    return _NC


def prep_in_maps(inputs):
    bf = ml_dtypes.bfloat16
    f32 = np.float32
    x = np.asarray(inputs["x"], f32)
    fm = np.asarray(inputs["forward_memory"], f32)
    rm = np.asarray(inputs["reverse_memory"], f32)
    Wq = np.asarray(inputs["Wq"], f32)
    Wk = np.asarray(inputs["Wk"], f32)
    Wv = np.asarray(inputs["Wv"], f32)
    Wo = np.asarray(inputs["Wo"], f32)
    gW = np.asarray(inputs["gate_W"], f32)
    gb = np.asarray(inputs["gate_b"], f32)
    cwf = np.asarray(inputs["canon_w"], f32)
    cbf = np.asarray(inputs["canon_bias"], f32)
    wg_eff = Wq @ gW  # gate reads q = x@Wq, so fold: x @ (Wq gate_W)
    scale = f32(1.0 / np.sqrt(D))
    in_maps = []
    for core in range(8):
        b, g = divmod(core, 4)
        cs = slice(g * CL, (g + 1) * CL)
        hs = slice(g * HL, (g + 1) * HL)
        xmT = np.ascontiguousarray(np.concatenate([x[b], fm[b], rm[b]], 0).T)
        in_maps.append({
            "xmT": xmT.astype(bf),
            "wq": np.ascontiguousarray(Wq[:, cs] * scale).astype(bf),
            "wk": np.ascontiguousarray(Wk[:, cs]).astype(bf),
            "wv": np.ascontiguousarray(Wv[:, cs]).astype(bf),
            "wo": np.ascontiguousarray(Wo[cs, :]).astype(bf),
            "wg": np.ascontiguousarray(wg_eff[:, hs]).astype(bf),
            "gb": np.ascontiguousarray(gb[hs]).reshape(HL, 1).astype(f32),
            "cw": np.ascontiguousarray(cwf[cs, 0, :]).astype(f32),
            "cb": np.ascontiguousarray(cbf[cs]).reshape(CL, 1).astype(f32),
        })
    return in_maps


def combine_results(parts):
    out = np.empty((B, T, C), np.float32)
    for b in range(B):
        out[b] = parts[4 * b] + parts[4 * b + 1] + parts[4 * b + 2] + parts[4 * b + 3]
    return out


def kernel(**inputs):
    nc = _get_nc()
    in_maps = prep_in_maps(inputs)
    res = run_bass_kernel_spmd(nc, in_maps, core_ids=list(range(8)))
    return combine_results([res.results[i]["out"] for i in range(8)])
